# revision 16
# baseline (speedup 1.0000x reference)
"""Trainium2 Bass kernel for nn_BMN (Boundary-Matching Network), 8-core SPMD.

Sharding: 8 cores = (batch b in {0,1}) x (start-index quarter iq in {0..3}).
Each core computes the full conv1d stack for its batch, the reassociated
boundary-matching matmul  out3d[o,(i,j)] = sum_{n,t} qT[(n,t),o] * mask[(n,t),(i,j)]
for its 25-row i-slice (+2 halo rows each side), then the 2D conv tower, and
writes cm[b, :, 25*iq:25*iq+25, :] plus (start, end) heads.

All matmuls run as float32r (full PE rate at free-dim >= 256, near-fp32
precision).  Biases are folded into matmuls as K=1 rows against a ones/validity
row, which also zeroes out-of-grid halo rows for free.
"""
import numpy as np

import concourse.bass as bass
import concourse.bacc as bacc
import concourse.mybir as mybir
import concourse.tile as tile
from concourse.bass_utils import run_bass_kernel_spmd

F32 = mybir.dt.float32
F32R = mybir.dt.float32r
F16 = mybir.dt.float16
AF = mybir.ActivationFunctionType
ALU = mybir.AluOpType

T, N, P, C_IN = 100, 32, 3, 400
H1, H2, H3, G, B = 256, 128, 512, 4, 2
NCORES = 8
RPC = 25            # output i-rows per core
REXT = 29           # extended rows (2-halo each side)
W = 102             # padded j width
FLAT = 2 + REXT * W + W
COLS = REXT * T     # 2900
CC = [(0, 500), (500, 500), (1000, 500), (1500, 500), (2000, 500), (2500, 400)]

# rows_sb offsets: packed bias rows [1, 2816]
R_B3D, R_BQ1 = 0, 512
R_BB1, R_BB2, R_BS1, R_BE1, R_BP = 640, 896, 1152, 1408, 1664
R_BQ2, R_BQ3, R_BQ4, R_BSE = 1920, 2048, 2176, 2178
R_ONES = 2304
ROWS_LEN = 2816
# fp16 bias rows (rowsh) offsets
H_B3D, H_BQ1, H_BQ2, H_BQ3, H_BQ4, H_ONES = 0, 512, 640, 768, 896, 1024
H_BB1, H_BB2, H_BS1, H_BE1, H_BP, H_BSE = 1536, 1792, 2048, 2304, 2560, 2816
ROWSH_LEN = 2848

DRAM_IN = [
    ("xp", [512, W]),
    ("wb1T", [128, 12 * 256]), ("wb2T", [128, 6 * 256]),
    ("ws1T", [128, 6 * 256]), ("we1T", [128, 6 * 256]), ("wpT", [128, 6 * 256]),
    ("wseT", [128, 4]),
    ("rowsh", [1, ROWSH_LEN]),
    ("valid", [1, COLS]),
    ("vmask2", [128, 27 * T]),
    ("w3dT", [2, 4, 128, 8 * H3]),
    ("maskf", [6, 5, 128, 2500]),
    ("wq1T", [128, 512]), ("wq2T", [128, 9 * 128]), ("wq3T", [128, 9 * 128]),
    ("wq4T", [128, 2]),
]
DRAM_OUT = [
    ("out_cm", [2, RPC * T]),
    ("out_start", [1, T]),
    ("out_end", [1, T]),
]


def _mm(nc, out, lhsT, rhs, start, stop):
    nc.tensor.matmul(out, lhsT, rhs, start=start, stop=stop)


def _Fi(r, jp):
    return 1 + r * W + jp


def _emit(tc, io):
    nc = tc.nc
    relu = nc.vector.tensor_relu

    with tc.tile_pool(name="main", bufs=1) as pm_sb:
        def ptile(tag, shape, dt=F16):
            return pm_sb.tile(shape, dt, tag=tag, name=tag)

        # ---- persistent tiles ----
        rowsh = ptile("rowsh", [1, ROWSH_LEN], F16)
        valid = ptile("valid", [1, COLS], F16)
        vmask2 = ptile("vmask2", [128, 27 * T], F32)
        pf = [ptile(f"pf{i}", [128, T], F16) for i in range(2)]
        qflat = [ptile(f"qf{k}", [128, H3], F16) for k in range(25)]
        x1 = ptile("x1", [128, FLAT], F16)
        x2 = ptile("x2", [128, FLAT], F16)
        wq1s = ptile("wq1s", [128, 512], F16)
        wq2s = ptile("wq2s", [128, 9 * 128], F16)
        wq3s = ptile("wq3s", [128, 9 * 128], F16)
        wq4s = ptile("wq4s", [128, 2], F16)
        ocm = ptile("ocm", [2, RPC * T], F32)
        ost = ptile("ost", [1, T], F32)
        oen = ptile("oen", [1, T], F32)

        nc.sync.dma_start(rowsh[:], io["rowsh"])
        nc.vector.memset(x1[:], 0.0)
        nc.vector.memset(x2[:], 0.0)

        onesh = rowsh[:, H_ONES:H_ONES + 512]

        # ================= phase A: conv1d stack + heads =================
        with tc.tile_pool(name="convw", bufs=1) as pw, \
             tc.tile_pool(name="convp", bufs=4, space="PSUM") as pp:

            def wtile(tag, shape):
                return pw.tile(shape, F16, tag=tag, name=tag)

            xp = [wtile(f"xp{k}", [128, W]) for k in range(4)]
            for kt in range(4):
                nc.sync.dma_start(xp[kt][:], io["xp"][kt * 128:(kt + 1) * 128, :])
            wb1 = wtile("wb1", [128, 12 * 256])
            nc.sync.dma_start(wb1[:], io["wb1T"])
            w2 = {}
            for nm in ("wb2T", "ws1T", "we1T", "wpT"):
                w2[nm] = wtile(nm, [128, 6 * 256])
                nc.sync.dma_start(w2[nm][:], io[nm])
            wse = wtile("wse", [128, 4])
            nc.sync.dma_start(wse[:], io["wseT"])

            h1 = [wtile(f"h1{i}", [128, W]) for i in range(2)]
            h = [wtile(f"h{i}", [128, W]) for i in range(2)]
            s_sb = [wtile(f"s{i}", [128, T]) for i in range(2)]
            e_sb = [wtile(f"e{i}", [128, T]) for i in range(2)]
            for i in range(2):
                nc.vector.memset(h1[i][:].bitcast(F32), 0.0)
                nc.vector.memset(h[i][:].bitcast(F32), 0.0)

            def conv1d(src_tiles, w_sb, nkt, bias_off, dst_tiles, dst_pad):
                for ot in range(2):
                    ps = pp.tile([128, T], F32, tag="cvp", name="cvp")
                    first = True
                    for k in range(3):
                        for kt in range(nkt):
                            lhsT = w_sb[:, (k * nkt + kt) * 256 + ot * 128:
                                        (k * nkt + kt) * 256 + ot * 128 + 128]
                            _mm(nc, ps[:], lhsT, src_tiles[kt][:, k:k + T],
                                first, False)
                            first = False
                    _mm(nc, ps[:], rowsh[:, bias_off + ot * 128:bias_off + ot * 128 + 128],
                        onesh[:, 0:T], False, True)
                    if dst_pad:
                        relu(dst_tiles[ot][:, 1:T + 1], ps[:])
                    else:
                        relu(dst_tiles[ot][:, 0:T], ps[:])

            conv1d(xp, wb1, 4, H_BB1, h1, True)
            conv1d(h1, w2["wb2T"], 2, H_BB2, h, True)
            conv1d(h, w2["ws1T"], 2, H_BS1, s_sb, False)
            conv1d(h, w2["we1T"], 2, H_BE1, e_sb, False)
            conv1d(h, w2["wpT"], 2, H_BP, pf, False)

            # heads: sigmoid(w . s + b)
            for col, src, dst in ((0, s_sb, ost), (1, e_sb, oen)):
                ph = pp.tile([1, T], F32, tag="cvh", name="cvh")
                for kt in range(2):
                    _mm(nc, ph[:], wse[:, kt * 2 + col:kt * 2 + col + 1],
                        src[kt][:, 0:T], kt == 0, False)
                _mm(nc, ph[:], rowsh[:, H_BSE + col:H_BSE + col + 1],
                    onesh[:, 0:T], False, True)
                nc.scalar.activation(dst[:], ph[:], AF.Sigmoid)
            nc.scalar.dma_start(io["out_start"], ost[:])
            nc.scalar.dma_start(io["out_end"], oen[:])

        # ================= phase B: qT + flat repack =================
        p_mk = tc.alloc_tile_pool(name="mpool", bufs=7)
        p_ps = tc.alloc_tile_pool(name="bcpsum", bufs=1, space="PSUM")
        mk_cc0 = []
        for ktg in range(5):
            mk = p_mk.tile([128, 2500], F16, tag="mk", name="mk")
            nc.gpsimd.dma_start(mk[:], io["maskf"][0, ktg])
            mk_cc0.append(mk)

        with tc.tile_pool(name="w3pool", bufs=3) as p_w3, \
             tc.tile_pool(name="qtmp", bufs=8) as p_qt:
            for g in range(4):
                w3a = p_w3.tile([128, 8 * H3], F16, tag="w3", name="w3a")
                nc.sync.dma_start(w3a[:], io["w3dT"][0, g])
                w3b = p_w3.tile([128, 8 * H3], F16, tag="w3", name="w3b")
                nc.sync.dma_start(w3b[:], io["w3dT"][1, g])
                for ni in range(8):
                    n = 8 * g + ni
                    qp = p_ps.tile([T, H3], F32, tag="qp", name="qp", bufs=2)
                    _mm(nc, qp[:], pf[0][:, 0:T],
                        w3a[:, ni * H3:(ni + 1) * H3], True, False)
                    _mm(nc, qp[:], pf[1][:, 0:T],
                        w3b[:, ni * H3:(ni + 1) * H3], False, True)
                    qt = p_qt.tile([T, H3], F16, tag="qt", name="qt")
                    nc.vector.tensor_copy(qt[:], qp[:])
                    # scatter rows [100n, 100n+100) into 128-row qflat tiles
                    g0 = n * T
                    r = 0
                    while r < T:
                        kt, off = divmod(g0 + r, 128)
                        cnt = min(T - r, 128 - off)
                        nc.gpsimd.dma_start(qflat[kt][off:off + cnt, :],
                                            qt[r:r + cnt, :])
                        r += cnt

        nc.sync.dma_start(valid[:], io["valid"])
        nc.sync.dma_start(vmask2[:], io["vmask2"])
        nc.sync.dma_start(wq1s[:], io["wq1T"])
        nc.sync.dma_start(wq2s[:], io["wq2T"])
        nc.sync.dma_start(wq3s[:], io["wq3T"])
        nc.sync.dma_start(wq4s[:], io["wq4T"])

        # ================= phase C: M2' + wq1, per column block =================
        with tc.tile_pool(name="ypool", bufs=2) as p_y:
            for cc, (c0, csz) in enumerate(CC):
                yt = []
                mps = [p_ps.tile([128, csz], F32, tag=f"mp{ot}", name=f"mp{ot}",
                                 bufs=1) for ot in range(4)]
                for ktg in range(5):
                    if cc == 0:
                        mk = mk_cc0[ktg]
                    else:
                        mk = p_mk.tile([128, 2500], F16, tag="mk", name="mk")
                        nc.sync.dma_start(mk[:], io["maskf"][cc, ktg])
                    for sub in range(5):
                        kt = 5 * ktg + sub
                        for ot in range(4):
                            _mm(nc, mps[ot][:],
                                qflat[kt][:, ot * 128:(ot + 1) * 128],
                                mk[:, sub * 500:sub * 500 + csz],
                                kt == 0, False)
                for ot in range(4):
                    _mm(nc, mps[ot][:], rowsh[:, H_B3D + ot * 128:H_B3D + ot * 128 + 128],
                        valid[:, c0:c0 + csz], False, True)
                    y = p_y.tile([128, 500], F16, tag=f"y{ot}", name=f"y{ot}")
                    relu(y[:, 0:csz], mps[ot][:])
                    yt.append(y)
                # wq1 1x1 -> x1 (padded strided dest)
                q1 = p_ps.tile([128, csz], F32, tag="q1", name="q1", bufs=1)
                for kt in range(4):
                    _mm(nc, q1[:], wq1s[:, kt * 128:(kt + 1) * 128],
                        yt[kt][:, 0:csz], kt == 0, False)
                _mm(nc, q1[:], rowsh[:, H_BQ1:H_BQ1 + 128],
                    valid[:, c0:c0 + csz], False, True)
                r0, nr = c0 // T, csz // T
                relu(x1[:, _Fi(r0, 1):_Fi(r0, 1) + nr * W]
                     .rearrange("p (r w) -> p r w", w=W)[:, :, 0:T],
                     q1[:].rearrange("p (r w) -> p r w", w=T))

        p_ps.release()
        p_mk.release()

        # ================= phase D: wq2 -> x2 (masked) =================
        with tc.tile_pool(name="c2psum", bufs=3, space="PSUM") as pc2, \
             tc.tile_pool(name="x3pool", bufs=2) as p_x3, \
             tc.tile_pool(name="c4psum", bufs=2, space="PSUM") as pc4:
            for r0, nr in ((1, 5), (6, 5), (11, 5), (16, 5), (21, 5), (26, 2)):
                wsz = nr * W
                ps = pc2.tile([128, 510], F32, tag="c2", name="c2")
                base = _Fi(r0, 0)
                for d in range(9):
                    di, dj = d // 3 - 1, d % 3 - 1
                    off = base + di * W + dj
                    _mm(nc, ps[:, 0:wsz], wq2s[:, d * 128:(d + 1) * 128],
                        x1[:, off:off + wsz], d == 0, False)
                _mm(nc, ps[:, 0:wsz], rowsh[:, H_BQ2:H_BQ2 + 128],
                    onesh[:, 0:wsz], False, True)
                # fused relu * validity-mask, strided into x2
                nc.vector.scalar_tensor_tensor(
                    x2[:, _Fi(r0, 1):_Fi(r0, 1) + wsz]
                    .rearrange("p (r w) -> p r w", w=W)[:, :, 0:T],
                    ps[:, 0:wsz].rearrange("p (r w) -> p r w", w=W)[:, :, 1:T + 1],
                    0.0,
                    vmask2[:, (r0 - 1) * T:(r0 - 1 + nr) * T]
                    .rearrange("p (r w) -> p r w", w=T),
                    ALU.max, ALU.mult)

            # ============= phase E: wq3 + wq4 + sigmoid out =============
            for bi, r0 in enumerate((2, 7, 12, 17, 22)):
                wsz = 5 * W
                ps = pc2.tile([128, 510], F32, tag="c2", name="c2")
                base = _Fi(r0, 0)
                for d in range(9):
                    di, dj = d // 3 - 1, d % 3 - 1
                    off = base + di * W + dj
                    _mm(nc, ps[:, 0:wsz], wq3s[:, d * 128:(d + 1) * 128],
                        x2[:, off:off + wsz], d == 0, False)
                _mm(nc, ps[:, 0:wsz], rowsh[:, H_BQ3:H_BQ3 + 128],
                    onesh[:, 0:wsz], False, True)
                x3 = p_x3.tile([128, 5 * T], F16, tag="x3", name="x3")
                relu(x3[:].rearrange("p (r w) -> p r w", w=T),
                     ps[:, 0:wsz].rearrange("p (r w) -> p r w", w=W)[:, :, 1:T + 1])
                p4 = pc4.tile([2, 5 * T], F32, tag="c4", name="c4")
                _mm(nc, p4[:], wq4s[:, 0:2], x3[:], True, False)
                _mm(nc, p4[:], rowsh[:, H_BQ4:H_BQ4 + 2], onesh[:, 0:5 * T],
                    False, True)
                nc.scalar.activation(ocm[:, bi * 5 * T:(bi + 1) * 5 * T], p4[:],
                                     AF.Sigmoid)
            nc.scalar.dma_start(io["out_cm"], ocm[:])


_CACHE = {}


def _build():
    if "nc" not in _CACHE:
        nc = bacc.Bacc("TRN2", target_bir_lowering=False, debug=False)
        io = {}
        for name, shape in DRAM_IN:
            dt = F32 if name == "vmask2" else F16
            io[name] = nc.dram_tensor(name, list(shape), dt,
                                      kind="ExternalInput").ap()
        for name, shape in DRAM_OUT:
            io[name] = nc.dram_tensor(name, list(shape), F32,
                                      kind="ExternalOutput").ap()
        with tile.TileContext(nc) as tc:
            _emit(tc, io)
        nc.compile()
        _CACHE["nc"] = nc
    return _CACHE["nc"]


# ---------------- host-side prep ----------------

def _dense_grouped(w, pad_to):
    # returns [128, (k kt) * 256] sbuf-image: lhsT slices at (k*nkt+kt)*256+o
    out_c, cin_g, K = w.shape
    og = out_c // G
    dense = np.zeros((K, pad_to, out_c), np.float16)
    for o in range(out_c):
        g = o // og
        dense[:, g * cin_g:(g + 1) * cin_g, o] = w[o].T
    nkt = pad_to // 128
    # [k, kt*128+p, o] -> [p, (k, kt), o]
    return np.ascontiguousarray(
        dense.reshape(K, nkt, 128, out_c).transpose(2, 0, 1, 3)
        .reshape(128, K * nkt * out_c))


def _host_prep(inputs):
    pr = {}
    pr["sm"] = np.ascontiguousarray(
        np.asarray(inputs["sample_mask"], np.float32).reshape(T, N, T, T))
    pr["wb1T"] = _dense_grouped(np.asarray(inputs["wb1"], np.float16), 512)
    pr["wb2T"] = _dense_grouped(np.asarray(inputs["wb2"], np.float16), 256)
    pr["ws1T"] = _dense_grouped(np.asarray(inputs["ws1"], np.float16), 256)
    pr["we1T"] = _dense_grouped(np.asarray(inputs["we1"], np.float16), 256)
    wpT = np.asarray(inputs["wp"], np.float16).transpose(2, 1, 0)  # [3,256,256]
    pr["wpT"] = np.ascontiguousarray(
        wpT.reshape(3, 2, 128, 256).transpose(2, 0, 1, 3).reshape(128, 6 * 256))
    wseT = np.stack([np.asarray(inputs["ws2"], np.float16)[0, :, 0],
                     np.asarray(inputs["we2"], np.float16)[0, :, 0]], axis=1)
    pr["wseT"] = np.ascontiguousarray(
        wseT.reshape(2, 128, 2).transpose(1, 0, 2).reshape(128, 4))
    rowsh = np.zeros((1, ROWSH_LEN), np.float16)
    rowsh[0, H_B3D:H_B3D + 512] = np.asarray(inputs["b3d"], np.float16)
    rowsh[0, H_BQ1:H_BQ1 + 128] = np.asarray(inputs["bq1"], np.float16)
    rowsh[0, H_BQ2:H_BQ2 + 128] = np.asarray(inputs["bq2"], np.float16)
    rowsh[0, H_BQ3:H_BQ3 + 128] = np.asarray(inputs["bq3"], np.float16)
    rowsh[0, H_BQ4:H_BQ4 + 2] = np.asarray(inputs["bq4"], np.float16)
    rowsh[0, H_ONES:H_ONES + 512] = 1.0
    for off, key in ((H_BB1, "bb1"), (H_BB2, "bb2"), (H_BS1, "bs1"),
                     (H_BE1, "be1"), (H_BP, "bp")):
        rowsh[0, off:off + 256] = np.asarray(inputs[key], np.float16)
    rowsh[0, H_BSE] = np.float16(np.asarray(inputs["bs2"])[0])
    rowsh[0, H_BSE + 1] = np.float16(np.asarray(inputs["be2"])[0])
    pr["rowsh"] = rowsh
    w3d = np.asarray(inputs["w3d"], np.float32)
    w3t = w3d.transpose(1, 2, 0).reshape(2, 128, 4, 8, H3)  # [ct,p,g,ni,o]
    pr["w3dT"] = np.ascontiguousarray(
        w3t.transpose(0, 2, 1, 3, 4).reshape(2, 4, 128, 8 * H3).astype(np.float16))
    wq1T = np.asarray(inputs["wq1"], np.float16)[:, :, 0, 0].T  # [512,128]
    pr["wq1T"] = np.ascontiguousarray(
        wq1T.reshape(4, 128, 128).transpose(1, 0, 2).reshape(128, 512))
    wq2T = np.asarray(inputs["wq2"], np.float16).transpose(2, 3, 1, 0).reshape(9, H2, H2)
    pr["wq2T"] = np.ascontiguousarray(
        wq2T.transpose(1, 0, 2).reshape(128, 9 * 128))
    wq3T = np.asarray(inputs["wq3"], np.float16).transpose(2, 3, 1, 0).reshape(9, H2, H2)
    pr["wq3T"] = np.ascontiguousarray(
        wq3T.transpose(1, 0, 2).reshape(128, 9 * 128))
    pr["wq4T"] = np.ascontiguousarray(
        np.asarray(inputs["wq4"], np.float16)[:, :, 0, 0].T)
    return pr


def _core_inputs(inputs, pr, core):
    b, iq = divmod(core, 4)
    lo = RPC * iq
    ci = {}
    xp = np.zeros((512, W), np.float16)
    xp[:C_IN, 1:T + 1] = np.asarray(inputs["x"], np.float16)[b].T
    ci["xp"] = xp
    msk = np.zeros((N, T, REXT, T), np.float32)
    rlo, rhi = max(0, 2 - lo), min(REXT, T + 2 - lo)
    msk[:, :, rlo:rhi, :] = pr["sm"][:, :, lo - 2 + rlo:lo - 2 + rhi, :] \
        .transpose(1, 0, 2, 3)
    mf = msk.reshape(N * T, REXT * T).reshape(25, 128, REXT * T)
    mg = np.zeros((6, 5, 128, 5, 500), np.float16)
    for cc, (c0, csz) in enumerate(CC):
        for ktg in range(5):
            mg[cc, ktg, :, :, :csz] = \
                mf[5 * ktg:5 * ktg + 5, :, c0:c0 + csz].transpose(1, 0, 2)
    ci["maskf"] = np.ascontiguousarray(mg.reshape(6, 5, 128, 2500))
    vr = np.zeros((REXT,), np.float32)
    vr[rlo:rhi] = 1.0
    ci["valid"] = np.repeat(vr, T)[None, :].astype(np.float16)
    ci["vmask2"] = np.broadcast_to(
        np.repeat(vr[1:28], T)[None, :], (128, 27 * T)).copy()
    for k in ("wb1T", "wb2T", "ws1T", "we1T", "wpT", "wseT", "rowsh",
              "w3dT", "wq1T", "wq2T", "wq3T", "wq4T"):
        ci[k] = pr[k]
    return ci


def _run(inputs, **spmd_kwargs):
    nc = _build()
    pr = _host_prep(inputs)
    in_maps = [_core_inputs(inputs, pr, c) for c in range(NCORES)]
    res = run_bass_kernel_spmd(nc, in_maps, core_ids=list(range(NCORES)),
                               **spmd_kwargs)
    cm = np.zeros((B, 2, T, T), np.float32)
    start = np.zeros((B, T), np.float32)
    end = np.zeros((B, T), np.float32)
    for core in range(NCORES):
        b, iq = divmod(core, 4)
        r = res.results[core]
        cm[b, :, RPC * iq:RPC * (iq + 1), :] = r["out_cm"].reshape(2, RPC, T)
        if iq == 0:
            start[b] = r["out_start"][0]
            end[b] = r["out_end"][0]
    return (cm, start, end), res


def kernel(**inputs):
    out, _ = _run(inputs)
    return out


# revision 17
# speedup vs baseline: 1.0288x; 1.0288x over previous
"""Trainium2 Bass kernel for nn_BMN (Boundary-Matching Network), 8-core SPMD.

Sharding: 8 cores = (batch b in {0,1}) x (start-index quarter iq in {0..3}).
Each core computes the full conv1d stack for its batch, the reassociated
boundary-matching matmul  out3d[o,(i,j)] = sum_{n,t} qT[(n,t),o] * mask[(n,t),(i,j)]
for its 25-row i-slice (+2 halo rows each side), then the 2D conv tower, and
writes cm[b, :, 25*iq:25*iq+25, :] plus (start, end) heads.

All matmuls run as float32r (full PE rate at free-dim >= 256, near-fp32
precision).  Biases are folded into matmuls as K=1 rows against a ones/validity
row, which also zeroes out-of-grid halo rows for free.
"""
import numpy as np

import concourse.bass as bass
import concourse.bacc as bacc
import concourse.mybir as mybir
import concourse.tile as tile
from concourse.bass_utils import run_bass_kernel_spmd

F32 = mybir.dt.float32
F32R = mybir.dt.float32r
F16 = mybir.dt.float16
AF = mybir.ActivationFunctionType
ALU = mybir.AluOpType

T, N, P, C_IN = 100, 32, 3, 400
H1, H2, H3, G, B = 256, 128, 512, 4, 2
NCORES = 8
RPC = 25            # output i-rows per core
REXT = 29           # extended rows (2-halo each side)
W = 102             # padded j width
FLAT = 2 + REXT * W + W
COLS = REXT * T     # 2900
CC = [(0, 500), (500, 500), (1000, 500), (1500, 500), (2000, 500), (2500, 400)]

# rows_sb offsets: packed bias rows [1, 2816]
R_B3D, R_BQ1 = 0, 512
R_BB1, R_BB2, R_BS1, R_BE1, R_BP = 640, 896, 1152, 1408, 1664
R_BQ2, R_BQ3, R_BQ4, R_BSE = 1920, 2048, 2176, 2178
R_ONES = 2304
ROWS_LEN = 2816
# fp16 bias rows (rowsh) offsets
H_B3D, H_BQ1, H_BQ2, H_BQ3, H_BQ4, H_ONES = 0, 512, 640, 768, 896, 1024
H_BB1, H_BB2, H_BS1, H_BE1, H_BP, H_BSE = 1536, 1792, 2048, 2304, 2560, 2816
ROWSH_LEN = 2848

DRAM_IN = [
    ("xp", [512, W]),
    ("wb1T", [128, 12 * 256]), ("wb2T", [128, 6 * 256]),
    ("ws1T", [128, 6 * 256]), ("we1T", [128, 6 * 256]), ("wpT", [128, 6 * 256]),
    ("wseT", [128, 4]),
    ("rowsh", [1, ROWSH_LEN]),
    ("valid", [1, COLS]),
    ("vmask2", [128, 27 * T]),
    ("w3dT", [2, 4, 128, 8 * H3]),
    ("maskf", [6, 5, 128, 2500]),
    ("wq1T", [128, 512]), ("wq2T", [128, 9 * 128]), ("wq3T", [128, 9 * 128]),
    ("wq4T", [128, 2]),
]
DRAM_OUT = [
    ("out_cm", [2, RPC * T]),
    ("out_start", [1, T]),
    ("out_end", [1, T]),
]


def _mm(nc, out, lhsT, rhs, start, stop):
    nc.tensor.matmul(out, lhsT, rhs, start=start, stop=stop)


def _Fi(r, jp):
    return 1 + r * W + jp


def _emit(tc, io):
    nc = tc.nc
    relu = nc.vector.tensor_relu

    with tc.tile_pool(name="main", bufs=1) as pm_sb:
        def ptile(tag, shape, dt=F16):
            return pm_sb.tile(shape, dt, tag=tag, name=tag)

        # ---- persistent tiles ----
        rowsh = ptile("rowsh", [1, ROWSH_LEN], F16)
        valid = ptile("valid", [1, COLS], F16)
        vmask2 = ptile("vmask2", [128, 27 * T], F32)
        pf = [ptile(f"pf{i}", [128, 128], F16) for i in range(2)]
        qflat = [ptile(f"qf{k}", [128, H3], F16) for k in range(25)]
        x1 = ptile("x1", [128, FLAT], F16)
        x2 = ptile("x2", [128, FLAT], F16)
        wq1s = ptile("wq1s", [128, 512], F16)
        wq2s = ptile("wq2s", [128, 9 * 128], F16)
        wq3s = ptile("wq3s", [128, 9 * 128], F16)
        wq4s = ptile("wq4s", [128, 2], F16)
        ocm = ptile("ocm", [2, RPC * T], F32)
        ost = ptile("ost", [1, T], F32)
        oen = ptile("oen", [1, T], F32)

        nc.sync.dma_start(rowsh[:], io["rowsh"])
        nc.vector.memset(x1[:], 0.0)
        nc.vector.memset(x2[:], 0.0)
        nc.vector.memset(pf[0][:], 0.0)
        nc.vector.memset(pf[1][:], 0.0)

        onesh = rowsh[:, H_ONES:H_ONES + 512]

        # ================= phase A: conv1d stack + heads =================
        with tc.tile_pool(name="convw", bufs=1) as pw, \
             tc.tile_pool(name="convp", bufs=4, space="PSUM") as pp:

            def wtile(tag, shape):
                return pw.tile(shape, F16, tag=tag, name=tag)

            xp = [wtile(f"xp{k}", [128, W]) for k in range(4)]
            for kt in range(4):
                nc.gpsimd.dma_start(xp[kt][:], io["xp"][kt * 128:(kt + 1) * 128, :])
            wb1 = wtile("wb1", [128, 12 * 256])
            nc.sync.dma_start(wb1[:], io["wb1T"])
            w2 = {}
            rings = [nc.scalar, nc.sync, nc.scalar, nc.sync]
            for ri, nm in enumerate(("wb2T", "ws1T", "we1T", "wpT")):
                w2[nm] = wtile(nm, [128, 6 * 256])
                rings[ri].dma_start(w2[nm][:], io[nm])
            wse = wtile("wse", [128, 4])
            nc.gpsimd.dma_start(wse[:], io["wseT"])

            h1 = [wtile(f"h1{i}", [128, W]) for i in range(2)]
            h = [wtile(f"h{i}", [128, W]) for i in range(2)]
            s_sb = [wtile(f"s{i}", [128, T]) for i in range(2)]
            e_sb = [wtile(f"e{i}", [128, T]) for i in range(2)]
            for i in range(2):
                nc.vector.memset(h1[i][:].bitcast(F32), 0.0)
                nc.vector.memset(h[i][:].bitcast(F32), 0.0)

            def conv1d(src_tiles, w_sb, nkt, bias_off, dst_tiles, dst_pad):
                for ot in range(2):
                    ps = pp.tile([128, T], F32, tag="cvp", name="cvp")
                    first = True
                    for k in range(3):
                        for kt in range(nkt):
                            lhsT = w_sb[:, (k * nkt + kt) * 256 + ot * 128:
                                        (k * nkt + kt) * 256 + ot * 128 + 128]
                            _mm(nc, ps[:], lhsT, src_tiles[kt][:, k:k + T],
                                first, False)
                            first = False
                    _mm(nc, ps[:], rowsh[:, bias_off + ot * 128:bias_off + ot * 128 + 128],
                        onesh[:, 0:T], False, True)
                    if dst_pad:
                        relu(dst_tiles[ot][:, 1:T + 1], ps[:])
                    else:
                        relu(dst_tiles[ot][:, 0:T], ps[:])

            conv1d(xp, wb1, 4, H_BB1, h1, True)
            conv1d(h1, w2["wb2T"], 2, H_BB2, h, True)
            conv1d(h, w2["ws1T"], 2, H_BS1, s_sb, False)
            conv1d(h, w2["we1T"], 2, H_BE1, e_sb, False)
            conv1d(h, w2["wpT"], 2, H_BP, pf, False)

            # heads: sigmoid(w . s + b)
            for col, src, dst in ((0, s_sb, ost), (1, e_sb, oen)):
                ph = pp.tile([1, T], F32, tag="cvh", name="cvh")
                for kt in range(2):
                    _mm(nc, ph[:], wse[:, kt * 2 + col:kt * 2 + col + 1],
                        src[kt][:, 0:T], kt == 0, False)
                _mm(nc, ph[:], rowsh[:, H_BSE + col:H_BSE + col + 1],
                    onesh[:, 0:T], False, True)
                nc.scalar.activation(dst[:], ph[:], AF.Sigmoid)
            nc.sync.dma_start(io["out_start"], ost[:])
            nc.sync.dma_start(io["out_end"], oen[:])

        # ================= phase B: qT + flat repack =================
        p_mk = tc.alloc_tile_pool(name="mpool", bufs=7)
        p_ps = tc.alloc_tile_pool(name="bcpsum", bufs=1, space="PSUM")
        mk_cc0 = []

        with tc.tile_pool(name="w3pool", bufs=3) as p_w3, \
             tc.tile_pool(name="qtmp", bufs=8) as p_qt:
            for g in range(4):
                w3a = p_w3.tile([128, 8 * H3], F16, tag="w3", name="w3a")
                nc.sync.dma_start(w3a[:], io["w3dT"][0, g])
                w3b = p_w3.tile([128, 8 * H3], F16, tag="w3", name="w3b")
                nc.sync.dma_start(w3b[:], io["w3dT"][1, g])
                for ktg in (range(2) if g == 0 else
                            range(2, 4) if g == 1 else
                            range(4, 5) if g == 2 else range(0)):
                    mk = p_mk.tile([128, 2500], F16, tag="mk", name="mk")
                    nc.sync.dma_start(mk[:], io["maskf"][0, ktg])
                    mk_cc0.append(mk)
                for ni in range(8):
                    n = 8 * g + ni
                    qp = p_ps.tile([128, H3], F32, tag="qp", name="qp", bufs=2)
                    _mm(nc, qp[:], pf[0][:], w3a[:, ni * H3:(ni + 1) * H3],
                        True, False)
                    _mm(nc, qp[:], pf[1][:], w3b[:, ni * H3:(ni + 1) * H3],
                        False, True)
                    qt = p_qt.tile([T, H3], F16, tag="qt", name="qt")
                    if n % 2 == 0:
                        nc.vector.tensor_copy(qt[:], qp[0:T, :])
                    else:
                        nc.scalar.copy(qt[:], qp[0:T, :])
                    # scatter rows [100n, 100n+100) into 128-row qflat tiles
                    g0 = n * T
                    r = 0
                    while r < T:
                        kt, off = divmod(g0 + r, 128)
                        cnt = min(T - r, 128 - off)
                        nc.gpsimd.dma_start(qflat[kt][off:off + cnt, :],
                                            qt[r:r + cnt, :])
                        r += cnt

        nc.sync.dma_start(valid[:], io["valid"])
        nc.sync.dma_start(vmask2[:], io["vmask2"])
        nc.sync.dma_start(wq1s[:], io["wq1T"])
        nc.sync.dma_start(wq2s[:], io["wq2T"])
        nc.sync.dma_start(wq3s[:], io["wq3T"])
        nc.sync.dma_start(wq4s[:], io["wq4T"])

        # ================= phase C: M2' + wq1, per column block =================
        with tc.tile_pool(name="ypool", bufs=2) as p_y:
            for cc, (c0, csz) in enumerate(CC):
                yt = []
                mps = [p_ps.tile([128, csz], F32, tag=f"mp{ot}", name=f"mp{ot}",
                                 bufs=1) for ot in range(4)]
                for ktg in range(5):
                    if cc == 0:
                        mk = mk_cc0[ktg]
                    else:
                        mk = p_mk.tile([128, 2500], F16, tag="mk", name="mk")
                        nc.sync.dma_start(mk[:], io["maskf"][cc, ktg])
                    for sub in range(5):
                        kt = 5 * ktg + sub
                        for ot in range(4):
                            _mm(nc, mps[ot][:],
                                qflat[kt][:, ot * 128:(ot + 1) * 128],
                                mk[:, sub * 500:sub * 500 + csz],
                                kt == 0, False)
                for ot in range(4):
                    _mm(nc, mps[ot][:], rowsh[:, H_B3D + ot * 128:H_B3D + ot * 128 + 128],
                        valid[:, c0:c0 + csz], False, True)
                    y = p_y.tile([128, 500], F16, tag=f"y{ot}", name=f"y{ot}")
                    relu(y[:, 0:csz], mps[ot][:])
                    yt.append(y)
                # wq1 1x1 -> x1 (padded strided dest)
                q1 = p_ps.tile([128, csz], F32, tag="q1", name="q1", bufs=1)
                for kt in range(4):
                    _mm(nc, q1[:], wq1s[:, kt * 128:(kt + 1) * 128],
                        yt[kt][:, 0:csz], kt == 0, False)
                _mm(nc, q1[:], rowsh[:, H_BQ1:H_BQ1 + 128],
                    valid[:, c0:c0 + csz], False, True)
                r0, nr = c0 // T, csz // T
                relu(x1[:, _Fi(r0, 1):_Fi(r0, 1) + nr * W]
                     .rearrange("p (r w) -> p r w", w=W)[:, :, 0:T],
                     q1[:].rearrange("p (r w) -> p r w", w=T))

        p_ps.release()
        p_mk.release()

        # ================= phase D: wq2 -> x2 (masked) =================
        with tc.tile_pool(name="c2psum", bufs=3, space="PSUM") as pc2, \
             tc.tile_pool(name="x3pool", bufs=2) as p_x3, \
             tc.tile_pool(name="c4psum", bufs=2, space="PSUM") as pc4:
            for r0, nr in ((1, 5), (6, 5), (11, 5), (16, 5), (21, 5), (26, 2)):
                wsz = nr * W
                ps = pc2.tile([128, 510], F32, tag="c2", name="c2")
                base = _Fi(r0, 0)
                for d in range(9):
                    di, dj = d // 3 - 1, d % 3 - 1
                    off = base + di * W + dj
                    _mm(nc, ps[:, 0:wsz], wq2s[:, d * 128:(d + 1) * 128],
                        x1[:, off:off + wsz], d == 0, False)
                _mm(nc, ps[:, 0:wsz], rowsh[:, H_BQ2:H_BQ2 + 128],
                    onesh[:, 0:wsz], False, True)
                # fused relu * validity-mask, strided into x2
                nc.vector.scalar_tensor_tensor(
                    x2[:, _Fi(r0, 1):_Fi(r0, 1) + wsz]
                    .rearrange("p (r w) -> p r w", w=W)[:, :, 0:T],
                    ps[:, 0:wsz].rearrange("p (r w) -> p r w", w=W)[:, :, 1:T + 1],
                    0.0,
                    vmask2[:, (r0 - 1) * T:(r0 - 1 + nr) * T]
                    .rearrange("p (r w) -> p r w", w=T),
                    ALU.max, ALU.mult)

            # ============= phase E: wq3 + wq4 + sigmoid out =============
            for bi, r0 in enumerate((2, 7, 12, 17, 22)):
                wsz = 5 * W
                ps = pc2.tile([128, 510], F32, tag="c2", name="c2")
                base = _Fi(r0, 0)
                for d in range(9):
                    di, dj = d // 3 - 1, d % 3 - 1
                    off = base + di * W + dj
                    _mm(nc, ps[:, 0:wsz], wq3s[:, d * 128:(d + 1) * 128],
                        x2[:, off:off + wsz], d == 0, False)
                _mm(nc, ps[:, 0:wsz], rowsh[:, H_BQ3:H_BQ3 + 128],
                    onesh[:, 0:wsz], False, True)
                x3 = p_x3.tile([128, 5 * T], F16, tag="x3", name="x3")
                relu(x3[:].rearrange("p (r w) -> p r w", w=T),
                     ps[:, 0:wsz].rearrange("p (r w) -> p r w", w=W)[:, :, 1:T + 1])
                p4 = pc4.tile([2, 5 * T], F32, tag="c4", name="c4")
                _mm(nc, p4[:], wq4s[:, 0:2], x3[:], True, False)
                _mm(nc, p4[:], rowsh[:, H_BQ4:H_BQ4 + 2], onesh[:, 0:5 * T],
                    False, True)
                nc.scalar.activation(ocm[:, bi * 5 * T:(bi + 1) * 5 * T], p4[:],
                                     AF.Sigmoid)
            nc.scalar.dma_start(io["out_cm"], ocm[:])


_CACHE = {}


def _build():
    if "nc" not in _CACHE:
        nc = bacc.Bacc("TRN2", target_bir_lowering=False, debug=False)
        io = {}
        for name, shape in DRAM_IN:
            dt = F32 if name == "vmask2" else F16
            io[name] = nc.dram_tensor(name, list(shape), dt,
                                      kind="ExternalInput").ap()
        for name, shape in DRAM_OUT:
            io[name] = nc.dram_tensor(name, list(shape), F32,
                                      kind="ExternalOutput").ap()
        with tile.TileContext(nc) as tc:
            _emit(tc, io)
        nc.compile()
        _CACHE["nc"] = nc
    return _CACHE["nc"]


# ---------------- host-side prep ----------------

def _dense_grouped(w, pad_to):
    # returns [128, (k kt) * 256] sbuf-image: lhsT slices at (k*nkt+kt)*256+o
    out_c, cin_g, K = w.shape
    og = out_c // G
    dense = np.zeros((K, pad_to, out_c), np.float16)
    for o in range(out_c):
        g = o // og
        dense[:, g * cin_g:(g + 1) * cin_g, o] = w[o].T
    nkt = pad_to // 128
    # [k, kt*128+p, o] -> [p, (k, kt), o]
    return np.ascontiguousarray(
        dense.reshape(K, nkt, 128, out_c).transpose(2, 0, 1, 3)
        .reshape(128, K * nkt * out_c))


def _host_prep(inputs):
    pr = {}
    pr["sm"] = np.ascontiguousarray(
        np.asarray(inputs["sample_mask"], np.float32).reshape(T, N, T, T))
    pr["wb1T"] = _dense_grouped(np.asarray(inputs["wb1"], np.float16), 512)
    pr["wb2T"] = _dense_grouped(np.asarray(inputs["wb2"], np.float16), 256)
    pr["ws1T"] = _dense_grouped(np.asarray(inputs["ws1"], np.float16), 256)
    pr["we1T"] = _dense_grouped(np.asarray(inputs["we1"], np.float16), 256)
    wpT = np.asarray(inputs["wp"], np.float16).transpose(2, 1, 0)  # [3,256,256]
    pr["wpT"] = np.ascontiguousarray(
        wpT.reshape(3, 2, 128, 256).transpose(2, 0, 1, 3).reshape(128, 6 * 256))
    wseT = np.stack([np.asarray(inputs["ws2"], np.float16)[0, :, 0],
                     np.asarray(inputs["we2"], np.float16)[0, :, 0]], axis=1)
    pr["wseT"] = np.ascontiguousarray(
        wseT.reshape(2, 128, 2).transpose(1, 0, 2).reshape(128, 4))
    rowsh = np.zeros((1, ROWSH_LEN), np.float16)
    rowsh[0, H_B3D:H_B3D + 512] = np.asarray(inputs["b3d"], np.float16)
    rowsh[0, H_BQ1:H_BQ1 + 128] = np.asarray(inputs["bq1"], np.float16)
    rowsh[0, H_BQ2:H_BQ2 + 128] = np.asarray(inputs["bq2"], np.float16)
    rowsh[0, H_BQ3:H_BQ3 + 128] = np.asarray(inputs["bq3"], np.float16)
    rowsh[0, H_BQ4:H_BQ4 + 2] = np.asarray(inputs["bq4"], np.float16)
    rowsh[0, H_ONES:H_ONES + 512] = 1.0
    for off, key in ((H_BB1, "bb1"), (H_BB2, "bb2"), (H_BS1, "bs1"),
                     (H_BE1, "be1"), (H_BP, "bp")):
        rowsh[0, off:off + 256] = np.asarray(inputs[key], np.float16)
    rowsh[0, H_BSE] = np.float16(np.asarray(inputs["bs2"])[0])
    rowsh[0, H_BSE + 1] = np.float16(np.asarray(inputs["be2"])[0])
    pr["rowsh"] = rowsh
    w3d = np.asarray(inputs["w3d"], np.float32)
    w3t = w3d.transpose(1, 2, 0).reshape(2, 128, 4, 8, H3)  # [ct,p,g,ni,o]
    pr["w3dT"] = np.ascontiguousarray(
        w3t.transpose(0, 2, 1, 3, 4).reshape(2, 4, 128, 8 * H3).astype(np.float16))
    wq1T = np.asarray(inputs["wq1"], np.float16)[:, :, 0, 0].T  # [512,128]
    pr["wq1T"] = np.ascontiguousarray(
        wq1T.reshape(4, 128, 128).transpose(1, 0, 2).reshape(128, 512))
    wq2T = np.asarray(inputs["wq2"], np.float16).transpose(2, 3, 1, 0).reshape(9, H2, H2)
    pr["wq2T"] = np.ascontiguousarray(
        wq2T.transpose(1, 0, 2).reshape(128, 9 * 128))
    wq3T = np.asarray(inputs["wq3"], np.float16).transpose(2, 3, 1, 0).reshape(9, H2, H2)
    pr["wq3T"] = np.ascontiguousarray(
        wq3T.transpose(1, 0, 2).reshape(128, 9 * 128))
    pr["wq4T"] = np.ascontiguousarray(
        np.asarray(inputs["wq4"], np.float16)[:, :, 0, 0].T)
    return pr


def _core_inputs(inputs, pr, core):
    b, iq = divmod(core, 4)
    lo = RPC * iq
    ci = {}
    xp = np.zeros((512, W), np.float16)
    xp[:C_IN, 1:T + 1] = np.asarray(inputs["x"], np.float16)[b].T
    ci["xp"] = xp
    msk = np.zeros((N, T, REXT, T), np.float32)
    rlo, rhi = max(0, 2 - lo), min(REXT, T + 2 - lo)
    msk[:, :, rlo:rhi, :] = pr["sm"][:, :, lo - 2 + rlo:lo - 2 + rhi, :] \
        .transpose(1, 0, 2, 3)
    mf = msk.reshape(N * T, REXT * T).reshape(25, 128, REXT * T)
    mg = np.zeros((6, 5, 128, 5, 500), np.float16)
    for cc, (c0, csz) in enumerate(CC):
        for ktg in range(5):
            mg[cc, ktg, :, :, :csz] = \
                mf[5 * ktg:5 * ktg + 5, :, c0:c0 + csz].transpose(1, 0, 2)
    ci["maskf"] = np.ascontiguousarray(mg.reshape(6, 5, 128, 2500))
    vr = np.zeros((REXT,), np.float32)
    vr[rlo:rhi] = 1.0
    ci["valid"] = np.repeat(vr, T)[None, :].astype(np.float16)
    ci["vmask2"] = np.broadcast_to(
        np.repeat(vr[1:28], T)[None, :], (128, 27 * T)).copy()
    for k in ("wb1T", "wb2T", "ws1T", "we1T", "wpT", "wseT", "rowsh",
              "w3dT", "wq1T", "wq2T", "wq3T", "wq4T"):
        ci[k] = pr[k]
    return ci


def _run(inputs, **spmd_kwargs):
    nc = _build()
    pr = _host_prep(inputs)
    in_maps = [_core_inputs(inputs, pr, c) for c in range(NCORES)]
    res = run_bass_kernel_spmd(nc, in_maps, core_ids=list(range(NCORES)),
                               **spmd_kwargs)
    cm = np.zeros((B, 2, T, T), np.float32)
    start = np.zeros((B, T), np.float32)
    end = np.zeros((B, T), np.float32)
    for core in range(NCORES):
        b, iq = divmod(core, 4)
        r = res.results[core]
        cm[b, :, RPC * iq:RPC * (iq + 1), :] = r["out_cm"].reshape(2, RPC, T)
        if iq == 0:
            start[b] = r["out_start"][0]
            end[b] = r["out_end"][0]
    return (cm, start, end), res


def kernel(**inputs):
    out, _ = _run(inputs)
    return out


# revision 18
# speedup vs baseline: 1.3192x; 1.2823x over previous
"""Trainium2 Bass kernel for nn_BMN (Boundary-Matching Network), 8-core SPMD.

Sharding: 8 cores = (batch b in {0,1}) x (start-index quarter iq in {0..3}).
Each core computes the full conv1d stack for its batch, the reassociated
boundary-matching matmul  out3d[o,(i,j)] = sum_{n,t} qT[(n,t),o] * mask[(n,t),(i,j)]
for its 25-row i-slice (+2 halo rows each side), then the 2D conv tower, and
writes cm[b, :, 25*iq:25*iq+25, :] plus (start, end) heads.

All matmuls run as float32r (full PE rate at free-dim >= 256, near-fp32
precision).  Biases are folded into matmuls as K=1 rows against a ones/validity
row, which also zeroes out-of-grid halo rows for free.
"""
import numpy as np
import ml_dtypes

import concourse.bass as bass
import concourse.bacc as bacc
import concourse.mybir as mybir
import concourse.tile as tile
from concourse.bass_utils import run_bass_kernel_spmd

F32 = mybir.dt.float32
F32R = mybir.dt.float32r
F16 = mybir.dt.float16
F8 = mybir.dt.float8e4
DR = mybir.MatmulPerfMode.DoubleRow
AF = mybir.ActivationFunctionType
ALU = mybir.AluOpType

T, N, P, C_IN = 100, 32, 3, 400
H1, H2, H3, G, B = 256, 128, 512, 4, 2
NCORES = 8
RPC = 25            # output i-rows per core
REXT = 29           # extended rows (2-halo each side)
W = 102             # padded j width
FLAT = 2 + REXT * W + W
COLS = REXT * T     # 2900
CC = [(0, 500), (500, 500), (1000, 500), (1500, 500), (2000, 500), (2500, 400)]

# rows_sb offsets: packed bias rows [1, 2816]
R_B3D, R_BQ1 = 0, 512
R_BB1, R_BB2, R_BS1, R_BE1, R_BP = 640, 896, 1152, 1408, 1664
R_BQ2, R_BQ3, R_BQ4, R_BSE = 1920, 2048, 2176, 2178
R_ONES = 2304
ROWS_LEN = 2816
# fp16 bias rows (rowsh) offsets
H_B3D, H_BQ1, H_BQ2, H_BQ3, H_BQ4, H_ONES = 0, 512, 640, 768, 896, 1024
H_BB1, H_BB2, H_BS1, H_BE1, H_BP, H_BSE = 1536, 1792, 2048, 2304, 2560, 2816
ROWSH_LEN = 2848

DRAM_IN = [
    ("xp", [512, W]),
    ("wb1T", [128, 12 * 256]), ("wb2T", [128, 6 * 256]),
    ("ws1T", [128, 6 * 256]), ("we1T", [128, 6 * 256]), ("wpT", [128, 6 * 256]),
    ("wseT", [128, 4]),
    ("rowsh", [1, ROWSH_LEN]),
    ("valid", [1, COLS]),
    ("vmask2", [128, 27 * T]),
    ("w3dT", [2, 4, 128, 8 * H3]),
    ("maskf", [6, 128, 13 * 2 * 512]),
    ("wq1T", [128, 512]), ("wq2T", [128, 9 * 128]), ("wq3T", [128, 9 * 128]),
    ("wq4T", [128, 2]),
]
DRAM_OUT = [
    ("out_cm", [2, RPC * T]),
    ("out_start", [1, T]),
    ("out_end", [1, T]),
]


def _mm(nc, out, lhsT, rhs, start, stop):
    nc.tensor.matmul(out, lhsT, rhs, start=start, stop=stop)


def _Fi(r, jp):
    return 1 + r * W + jp


def _emit(tc, io):
    nc = tc.nc
    relu = nc.vector.tensor_relu

    with tc.tile_pool(name="main", bufs=1) as pm_sb:
        def ptile(tag, shape, dt=F16):
            return pm_sb.tile(shape, dt, tag=tag, name=tag)

        # ---- persistent tiles ----
        rowsh = ptile("rowsh", [1, ROWSH_LEN], F16)
        valid = ptile("valid", [1, COLS], F16)
        vmask2 = ptile("vmask2", [128, 27 * T], F32)
        pf = [ptile(f"pf{i}", [128, 128], F16) for i in range(2)]
        qflat = [ptile(f"qf{k}", [128, 2 * H3], F8) for k in range(13)]
        x1 = ptile("x1", [128, FLAT], F16)
        x2 = ptile("x2", [128, FLAT], F16)
        wq1s = ptile("wq1s", [128, 512], F16)
        wq2s = ptile("wq2s", [128, 9 * 128], F16)
        wq3s = ptile("wq3s", [128, 9 * 128], F16)
        wq4s = ptile("wq4s", [128, 2], F16)
        ocm = ptile("ocm", [2, RPC * T], F32)
        ost = ptile("ost", [1, T], F32)
        oen = ptile("oen", [1, T], F32)

        nc.sync.dma_start(rowsh[:], io["rowsh"])
        nc.vector.memset(x1[:], 0.0)
        nc.vector.memset(x2[:], 0.0)
        nc.vector.memset(pf[0][:], 0.0)
        nc.vector.memset(pf[1][:], 0.0)
        nc.vector.memset(qflat[12][:].bitcast(F32), 0.0)

        onesh = rowsh[:, H_ONES:H_ONES + 512]

        # ================= phase A: conv1d stack + heads =================
        with tc.tile_pool(name="convw", bufs=1) as pw, \
             tc.tile_pool(name="convp", bufs=4, space="PSUM") as pp:

            def wtile(tag, shape):
                return pw.tile(shape, F16, tag=tag, name=tag)

            xp = [wtile(f"xp{k}", [128, W]) for k in range(4)]
            for kt in range(4):
                nc.gpsimd.dma_start(xp[kt][:], io["xp"][kt * 128:(kt + 1) * 128, :])
            wb1 = wtile("wb1", [128, 12 * 256])
            nc.sync.dma_start(wb1[:], io["wb1T"])
            w2 = {}
            rings = [nc.scalar, nc.sync, nc.scalar, nc.sync]
            for ri, nm in enumerate(("wb2T", "ws1T", "we1T", "wpT")):
                w2[nm] = wtile(nm, [128, 6 * 256])
                rings[ri].dma_start(w2[nm][:], io[nm])
            wse = wtile("wse", [128, 4])
            nc.gpsimd.dma_start(wse[:], io["wseT"])

            h1 = [wtile(f"h1{i}", [128, W]) for i in range(2)]
            h = [wtile(f"h{i}", [128, W]) for i in range(2)]
            s_sb = [wtile(f"s{i}", [128, T]) for i in range(2)]
            e_sb = [wtile(f"e{i}", [128, T]) for i in range(2)]
            for i in range(2):
                nc.vector.memset(h1[i][:].bitcast(F32), 0.0)
                nc.vector.memset(h[i][:].bitcast(F32), 0.0)

            def conv1d(src_tiles, w_sb, nkt, bias_off, dst_tiles, dst_pad):
                for ot in range(2):
                    ps = pp.tile([128, T], F32, tag="cvp", name="cvp")
                    first = True
                    for k in range(3):
                        for kt in range(nkt):
                            lhsT = w_sb[:, (k * nkt + kt) * 256 + ot * 128:
                                        (k * nkt + kt) * 256 + ot * 128 + 128]
                            _mm(nc, ps[:], lhsT, src_tiles[kt][:, k:k + T],
                                first, False)
                            first = False
                    _mm(nc, ps[:], rowsh[:, bias_off + ot * 128:bias_off + ot * 128 + 128],
                        onesh[:, 0:T], False, True)
                    if dst_pad:
                        relu(dst_tiles[ot][:, 1:T + 1], ps[:])
                    else:
                        relu(dst_tiles[ot][:, 0:T], ps[:])

            conv1d(xp, wb1, 4, H_BB1, h1, True)
            conv1d(h1, w2["wb2T"], 2, H_BB2, h, True)
            conv1d(h, w2["ws1T"], 2, H_BS1, s_sb, False)
            conv1d(h, w2["we1T"], 2, H_BE1, e_sb, False)
            conv1d(h, w2["wpT"], 2, H_BP, pf, False)

            # heads: sigmoid(w . s + b)
            for col, src, dst in ((0, s_sb, ost), (1, e_sb, oen)):
                ph = pp.tile([1, T], F32, tag="cvh", name="cvh")
                for kt in range(2):
                    _mm(nc, ph[:], wse[:, kt * 2 + col:kt * 2 + col + 1],
                        src[kt][:, 0:T], kt == 0, False)
                _mm(nc, ph[:], rowsh[:, H_BSE + col:H_BSE + col + 1],
                    onesh[:, 0:T], False, True)
                nc.scalar.activation(dst[:], ph[:], AF.Sigmoid)
            nc.sync.dma_start(io["out_start"], ost[:])
            nc.sync.dma_start(io["out_end"], oen[:])

        # ================= phase B: qT + flat repack =================
        p_mk = tc.alloc_tile_pool(name="mpool", bufs=7)
        p_ps = tc.alloc_tile_pool(name="bcpsum", bufs=1, space="PSUM")
        mk_cc0 = []

        with tc.tile_pool(name="w3pool", bufs=3) as p_w3, \
             tc.tile_pool(name="qtmp", bufs=8) as p_qt:
            for g in range(4):
                w3a = p_w3.tile([128, 8 * H3], F16, tag="w3", name="w3a")
                nc.sync.dma_start(w3a[:], io["w3dT"][0, g])
                w3b = p_w3.tile([128, 8 * H3], F16, tag="w3", name="w3b")
                nc.sync.dma_start(w3b[:], io["w3dT"][1, g])
                for (a, b) in ([(0, 5)] if g == 0 else
                               [(5, 9)] if g == 1 else
                               [(9, 13)] if g == 2 else []):
                    mk = p_mk.tile([128, 5 * 1024], F8, tag="mk", name="mk")
                    nc.sync.dma_start(
                        mk[:, 0:(b - a) * 1024],
                        io["maskf"][0, :, a * 1024:b * 1024])
                    mk_cc0.append(mk)
                for ni in range(8):
                    n = 8 * g + ni
                    qp = p_ps.tile([128, H3], F32, tag="qp", name="qp", bufs=2)
                    _mm(nc, qp[:], pf[0][:], w3a[:, ni * H3:(ni + 1) * H3],
                        True, False)
                    _mm(nc, qp[:], pf[1][:], w3b[:, ni * H3:(ni + 1) * H3],
                        False, True)
                    qt = p_qt.tile([T, H3], F8, tag="qt", name="qt")
                    if n % 2 == 0:
                        nc.vector.tensor_copy(qt[:], qp[0:T, :])
                    else:
                        nc.scalar.copy(qt[:], qp[0:T, :])
                    # scatter rows [100n, 100n+100) into (chain, plane) tiles
                    g0 = n * T
                    r = 0
                    while r < T:
                        kt, off = divmod(g0 + r, 128)
                        kt2, plane = divmod(kt, 2)
                        cnt = min(T - r, 128 - off)
                        nc.gpsimd.dma_start(
                            qflat[kt2][off:off + cnt,
                                       plane * H3:plane * H3 + H3],
                            qt[r:r + cnt, :])
                        r += cnt

        nc.sync.dma_start(valid[:], io["valid"])
        nc.sync.dma_start(vmask2[:], io["vmask2"])
        nc.sync.dma_start(wq1s[:], io["wq1T"])
        nc.sync.dma_start(wq2s[:], io["wq2T"])
        nc.sync.dma_start(wq3s[:], io["wq3T"])
        nc.sync.dma_start(wq4s[:], io["wq4T"])

        # ================= phase C: M2' + wq1, per column block =================
        with tc.tile_pool(name="ypool", bufs=2) as p_y:
            for cc, (c0, csz) in enumerate(CC):
                yt = []
                mps = [p_ps.tile([128, csz], F32, tag=f"mp{ot}", name=f"mp{ot}",
                                 bufs=1) for ot in range(4)]
                for gi, (a, b) in enumerate(((0, 5), (5, 9), (9, 13))):
                    if cc == 0:
                        mk = mk_cc0[gi]
                    else:
                        mk = p_mk.tile([128, 5 * 1024], F8, tag="mk", name="mk")
                        nc.sync.dma_start(
                            mk[:, 0:(b - a) * 1024],
                            io["maskf"][cc, :, a * 1024:b * 1024])
                    mkv = mk[:].rearrange("p (l two c) -> p l two c",
                                          two=2, c=512)
                    for sub in range(b - a):
                        kt2 = a + sub
                        rhs = mkv[:, sub, :, 0:csz]
                        for ot in range(4):
                            qv = qflat[kt2][:].rearrange(
                                "p (two o) -> p two o", two=2)
                            nc.tensor.matmul(
                                mps[ot][:], qv[:, :, ot * 128:(ot + 1) * 128],
                                rhs, start=(kt2 == 0), stop=False,
                                perf_mode=DR)
                for ot in range(4):
                    _mm(nc, mps[ot][:], rowsh[:, H_B3D + ot * 128:H_B3D + ot * 128 + 128],
                        valid[:, c0:c0 + csz], False, True)
                    y = p_y.tile([128, 500], F16, tag=f"y{ot}", name=f"y{ot}")
                    relu(y[:, 0:csz], mps[ot][:])
                    yt.append(y)
                # wq1 1x1 -> x1 (padded strided dest)
                q1 = p_ps.tile([128, csz], F32, tag="q1", name="q1", bufs=1)
                for kt in range(4):
                    _mm(nc, q1[:], wq1s[:, kt * 128:(kt + 1) * 128],
                        yt[kt][:, 0:csz], kt == 0, False)
                _mm(nc, q1[:], rowsh[:, H_BQ1:H_BQ1 + 128],
                    valid[:, c0:c0 + csz], False, True)
                r0, nr = c0 // T, csz // T
                relu(x1[:, _Fi(r0, 1):_Fi(r0, 1) + nr * W]
                     .rearrange("p (r w) -> p r w", w=W)[:, :, 0:T],
                     q1[:].rearrange("p (r w) -> p r w", w=T))

        p_ps.release()
        p_mk.release()

        # ================= phase D: wq2 -> x2 (masked) =================
        with tc.tile_pool(name="c2psum", bufs=3, space="PSUM") as pc2, \
             tc.tile_pool(name="x3pool", bufs=2) as p_x3, \
             tc.tile_pool(name="c4psum", bufs=2, space="PSUM") as pc4:
            for r0, nr in ((1, 5), (6, 5), (11, 5), (16, 5), (21, 5), (26, 2)):
                wsz = nr * W
                ps = pc2.tile([128, 510], F32, tag="c2", name="c2")
                base = _Fi(r0, 0)
                for d in range(9):
                    di, dj = d // 3 - 1, d % 3 - 1
                    off = base + di * W + dj
                    _mm(nc, ps[:, 0:wsz], wq2s[:, d * 128:(d + 1) * 128],
                        x1[:, off:off + wsz], d == 0, False)
                _mm(nc, ps[:, 0:wsz], rowsh[:, H_BQ2:H_BQ2 + 128],
                    onesh[:, 0:wsz], False, True)
                # fused relu * validity-mask, strided into x2
                nc.vector.scalar_tensor_tensor(
                    x2[:, _Fi(r0, 1):_Fi(r0, 1) + wsz]
                    .rearrange("p (r w) -> p r w", w=W)[:, :, 0:T],
                    ps[:, 0:wsz].rearrange("p (r w) -> p r w", w=W)[:, :, 1:T + 1],
                    0.0,
                    vmask2[:, (r0 - 1) * T:(r0 - 1 + nr) * T]
                    .rearrange("p (r w) -> p r w", w=T),
                    ALU.max, ALU.mult)

            # ============= phase E: wq3 + wq4 + sigmoid out =============
            for bi, r0 in enumerate((2, 7, 12, 17, 22)):
                wsz = 5 * W
                ps = pc2.tile([128, 510], F32, tag="c2", name="c2")
                base = _Fi(r0, 0)
                for d in range(9):
                    di, dj = d // 3 - 1, d % 3 - 1
                    off = base + di * W + dj
                    _mm(nc, ps[:, 0:wsz], wq3s[:, d * 128:(d + 1) * 128],
                        x2[:, off:off + wsz], d == 0, False)
                _mm(nc, ps[:, 0:wsz], rowsh[:, H_BQ3:H_BQ3 + 128],
                    onesh[:, 0:wsz], False, True)
                x3 = p_x3.tile([128, 5 * T], F16, tag="x3", name="x3")
                relu(x3[:].rearrange("p (r w) -> p r w", w=T),
                     ps[:, 0:wsz].rearrange("p (r w) -> p r w", w=W)[:, :, 1:T + 1])
                p4 = pc4.tile([2, 5 * T], F32, tag="c4", name="c4")
                _mm(nc, p4[:], wq4s[:, 0:2], x3[:], True, False)
                _mm(nc, p4[:], rowsh[:, H_BQ4:H_BQ4 + 2], onesh[:, 0:5 * T],
                    False, True)
                nc.scalar.activation(ocm[:, bi * 5 * T:(bi + 1) * 5 * T], p4[:],
                                     AF.Sigmoid)
            nc.scalar.dma_start(io["out_cm"], ocm[:])


_CACHE = {}


def _build():
    if "nc" not in _CACHE:
        nc = bacc.Bacc("TRN2", target_bir_lowering=False, debug=False)
        io = {}
        for name, shape in DRAM_IN:
            dt = F32 if name == "vmask2" else (F8 if name == "maskf" else F16)
            io[name] = nc.dram_tensor(name, list(shape), dt,
                                      kind="ExternalInput").ap()
        for name, shape in DRAM_OUT:
            io[name] = nc.dram_tensor(name, list(shape), F32,
                                      kind="ExternalOutput").ap()
        with tile.TileContext(nc) as tc:
            _emit(tc, io)
        nc.compile()
        _CACHE["nc"] = nc
    return _CACHE["nc"]


# ---------------- host-side prep ----------------

def _dense_grouped(w, pad_to):
    # returns [128, (k kt) * 256] sbuf-image: lhsT slices at (k*nkt+kt)*256+o
    out_c, cin_g, K = w.shape
    og = out_c // G
    dense = np.zeros((K, pad_to, out_c), np.float16)
    for o in range(out_c):
        g = o // og
        dense[:, g * cin_g:(g + 1) * cin_g, o] = w[o].T
    nkt = pad_to // 128
    # [k, kt*128+p, o] -> [p, (k, kt), o]
    return np.ascontiguousarray(
        dense.reshape(K, nkt, 128, out_c).transpose(2, 0, 1, 3)
        .reshape(128, K * nkt * out_c))


def _host_prep(inputs):
    pr = {}
    pr["sm"] = np.ascontiguousarray(
        np.asarray(inputs["sample_mask"], np.float32).reshape(T, N, T, T))
    pr["wb1T"] = _dense_grouped(np.asarray(inputs["wb1"], np.float16), 512)
    pr["wb2T"] = _dense_grouped(np.asarray(inputs["wb2"], np.float16), 256)
    pr["ws1T"] = _dense_grouped(np.asarray(inputs["ws1"], np.float16), 256)
    pr["we1T"] = _dense_grouped(np.asarray(inputs["we1"], np.float16), 256)
    wpT = np.asarray(inputs["wp"], np.float16).transpose(2, 1, 0)  # [3,256,256]
    pr["wpT"] = np.ascontiguousarray(
        wpT.reshape(3, 2, 128, 256).transpose(2, 0, 1, 3).reshape(128, 6 * 256))
    wseT = np.stack([np.asarray(inputs["ws2"], np.float16)[0, :, 0],
                     np.asarray(inputs["we2"], np.float16)[0, :, 0]], axis=1)
    pr["wseT"] = np.ascontiguousarray(
        wseT.reshape(2, 128, 2).transpose(1, 0, 2).reshape(128, 4))
    rowsh = np.zeros((1, ROWSH_LEN), np.float16)
    rowsh[0, H_B3D:H_B3D + 512] = np.asarray(inputs["b3d"], np.float16)
    rowsh[0, H_BQ1:H_BQ1 + 128] = np.asarray(inputs["bq1"], np.float16)
    rowsh[0, H_BQ2:H_BQ2 + 128] = np.asarray(inputs["bq2"], np.float16)
    rowsh[0, H_BQ3:H_BQ3 + 128] = np.asarray(inputs["bq3"], np.float16)
    rowsh[0, H_BQ4:H_BQ4 + 2] = np.asarray(inputs["bq4"], np.float16)
    rowsh[0, H_ONES:H_ONES + 512] = 1.0
    for off, key in ((H_BB1, "bb1"), (H_BB2, "bb2"), (H_BS1, "bs1"),
                     (H_BE1, "be1"), (H_BP, "bp")):
        rowsh[0, off:off + 256] = np.asarray(inputs[key], np.float16)
    rowsh[0, H_BSE] = np.float16(np.asarray(inputs["bs2"])[0])
    rowsh[0, H_BSE + 1] = np.float16(np.asarray(inputs["be2"])[0])
    pr["rowsh"] = rowsh
    w3d = np.asarray(inputs["w3d"], np.float32)
    w3t = w3d.transpose(1, 2, 0).reshape(2, 128, 4, 8, H3)  # [ct,p,g,ni,o]
    pr["w3dT"] = np.ascontiguousarray(
        w3t.transpose(0, 2, 1, 3, 4).reshape(2, 4, 128, 8 * H3).astype(np.float16))
    wq1T = np.asarray(inputs["wq1"], np.float16)[:, :, 0, 0].T  # [512,128]
    pr["wq1T"] = np.ascontiguousarray(
        wq1T.reshape(4, 128, 128).transpose(1, 0, 2).reshape(128, 512))
    wq2T = np.asarray(inputs["wq2"], np.float16).transpose(2, 3, 1, 0).reshape(9, H2, H2)
    pr["wq2T"] = np.ascontiguousarray(
        wq2T.transpose(1, 0, 2).reshape(128, 9 * 128))
    wq3T = np.asarray(inputs["wq3"], np.float16).transpose(2, 3, 1, 0).reshape(9, H2, H2)
    pr["wq3T"] = np.ascontiguousarray(
        wq3T.transpose(1, 0, 2).reshape(128, 9 * 128))
    pr["wq4T"] = np.ascontiguousarray(
        np.asarray(inputs["wq4"], np.float16)[:, :, 0, 0].T)
    return pr


def _core_inputs(inputs, pr, core):
    b, iq = divmod(core, 4)
    lo = RPC * iq
    ci = {}
    xp = np.zeros((512, W), np.float16)
    xp[:C_IN, 1:T + 1] = np.asarray(inputs["x"], np.float16)[b].T
    ci["xp"] = xp
    msk = np.zeros((N, T, REXT, T), np.float32)
    rlo, rhi = max(0, 2 - lo), min(REXT, T + 2 - lo)
    msk[:, :, rlo:rhi, :] = pr["sm"][:, :, lo - 2 + rlo:lo - 2 + rhi, :] \
        .transpose(1, 0, 2, 3)
    np8 = mybir.dt.np(F8)
    mf = np.zeros((13, 2, 128, REXT * T), np.float32)
    mf.reshape(13 * 2 * 128, REXT * T)[:N * T] = msk.reshape(N * T, REXT * T)
    mg = np.zeros((6, 128, 13, 2, 512), np8)
    for cc, (c0, csz) in enumerate(CC):
        mg[cc, :, :, :, :csz] = \
            mf[:, :, :, c0:c0 + csz].transpose(2, 0, 1, 3).astype(np8)
    ci["maskf"] = np.ascontiguousarray(mg.reshape(6, 128, 13 * 2 * 512))
    vr = np.zeros((REXT,), np.float32)
    vr[rlo:rhi] = 1.0
    ci["valid"] = np.repeat(vr, T)[None, :].astype(np.float16)
    ci["vmask2"] = np.broadcast_to(
        np.repeat(vr[1:28], T)[None, :], (128, 27 * T)).copy()
    for k in ("wb1T", "wb2T", "ws1T", "we1T", "wpT", "wseT", "rowsh",
              "w3dT", "wq1T", "wq2T", "wq3T", "wq4T"):
        ci[k] = pr[k]
    return ci


def _run(inputs, **spmd_kwargs):
    nc = _build()
    pr = _host_prep(inputs)
    in_maps = [_core_inputs(inputs, pr, c) for c in range(NCORES)]
    res = run_bass_kernel_spmd(nc, in_maps, core_ids=list(range(NCORES)),
                               **spmd_kwargs)
    cm = np.zeros((B, 2, T, T), np.float32)
    start = np.zeros((B, T), np.float32)
    end = np.zeros((B, T), np.float32)
    for core in range(NCORES):
        b, iq = divmod(core, 4)
        r = res.results[core]
        cm[b, :, RPC * iq:RPC * (iq + 1), :] = r["out_cm"].reshape(2, RPC, T)
        if iq == 0:
            start[b] = r["out_start"][0]
            end[b] = r["out_end"][0]
    return (cm, start, end), res


def kernel(**inputs):
    out, _ = _run(inputs)
    return out


# revision 19
# speedup vs baseline: 1.4732x; 1.1167x over previous
"""Trainium2 Bass kernel for nn_BMN (Boundary-Matching Network), 8-core SPMD.

Sharding: 8 cores = (batch b in {0,1}) x (start-index quarter iq in {0..3}).
Each core computes the full conv1d stack for its batch, the reassociated
boundary-matching matmul  out3d[o,(i,j)] = sum_{n,t} qT[(n,t),o] * mask[(n,t),(i,j)]
for its 25-row i-slice (+2 halo rows each side), then the 2D conv tower, and
writes cm[b, :, 25*iq:25*iq+25, :] plus (start, end) heads.

All matmuls run as float32r (full PE rate at free-dim >= 256, near-fp32
precision).  Biases are folded into matmuls as K=1 rows against a ones/validity
row, which also zeroes out-of-grid halo rows for free.
"""
import numpy as np
import ml_dtypes

import concourse.bass as bass
import concourse.bacc as bacc
import concourse.mybir as mybir
import concourse.tile as tile
from concourse.bass_utils import run_bass_kernel_spmd

F32 = mybir.dt.float32
F32R = mybir.dt.float32r
F16 = mybir.dt.float16
F8 = mybir.dt.float8e4
DR = mybir.MatmulPerfMode.DoubleRow
AF = mybir.ActivationFunctionType
ALU = mybir.AluOpType

T, N, P, C_IN = 100, 32, 3, 400
H1, H2, H3, G, B = 256, 128, 512, 4, 2
NCORES = 8
RPC = 25            # output i-rows per core
REXT = 29           # extended rows (2-halo each side)
W = 102             # padded j width
FLAT = 2 + REXT * W + W
COLS = REXT * T     # 2900
CC = [(0, 500), (500, 500), (1000, 500), (1500, 500), (2000, 500), (2500, 400)]

# rows_sb offsets: packed bias rows [1, 2816]
R_B3D, R_BQ1 = 0, 512
R_BB1, R_BB2, R_BS1, R_BE1, R_BP = 640, 896, 1152, 1408, 1664
R_BQ2, R_BQ3, R_BQ4, R_BSE = 1920, 2048, 2176, 2178
R_ONES = 2304
ROWS_LEN = 2816
# fp16 bias rows (rowsh) offsets
H_B3D, H_BQ1, H_BQ2, H_BQ3, H_BQ4, H_ONES = 0, 512, 640, 768, 896, 1024
H_BB1, H_BB2, H_BS1, H_BE1, H_BP, H_BSE = 1536, 1792, 2048, 2304, 2560, 2816
ROWSH_LEN = 2848

DRAM_IN = [
    ("xp", [512, W]),
    ("wb1T", [128, 12 * 256]), ("wb2T", [128, 6 * 256]),
    ("ws1T", [128, 6 * 256]), ("we1T", [128, 6 * 256]), ("wpT", [128, 6 * 256]),
    ("wseT", [128, 4]),
    ("rowsh", [1, ROWSH_LEN]),
    ("valid", [1, COLS]),
    ("vmask2", [128, 27 * T]),
    ("w3dT", [4, 128, 16 * H3]),
    ("maskf", [6, 128, 13 * 2 * 512]),
    ("wq1T", [128, 512]), ("wq2T", [128, 9 * 128]), ("wq3T", [128, 9 * 128]),
    ("wq4T", [128, 2]),
]
DRAM_OUT = [
    ("out_cm", [2, RPC * T]),
    ("out_start", [1, T]),
    ("out_end", [1, T]),
]


def _mm(nc, out, lhsT, rhs, start, stop):
    nc.tensor.matmul(out, lhsT, rhs, start=start, stop=stop)


def _Fi(r, jp):
    return 1 + r * W + jp


def _emit(tc, io):
    nc = tc.nc
    relu = nc.vector.tensor_relu

    with tc.tile_pool(name="main", bufs=1) as pm_sb:
        def ptile(tag, shape, dt=F16):
            return pm_sb.tile(shape, dt, tag=tag, name=tag)

        # ---- persistent tiles ----
        rowsh = ptile("rowsh", [1, ROWSH_LEN], F16)
        valid = ptile("valid", [1, COLS], F16)
        vmask2 = ptile("vmask2", [128, 27 * T], F32)
        pf = ptile("pf", [128, 2 * 128], F8)
        qflat = [ptile(f"qf{k}", [128, 2 * H3], F8) for k in range(13)]
        x1 = ptile("x1", [128, FLAT], F16)
        x2 = ptile("x2", [128, FLAT], F16)
        wq1s = ptile("wq1s", [128, 512], F16)
        wq2s = ptile("wq2s", [128, 9 * 128], F16)
        wq3s = ptile("wq3s", [128, 9 * 128], F16)
        wq4s = ptile("wq4s", [128, 2], F16)
        ocm = ptile("ocm", [2, RPC * T], F32)
        ost = ptile("ost", [1, T], F32)
        oen = ptile("oen", [1, T], F32)

        nc.sync.dma_start(rowsh[:], io["rowsh"])
        nc.vector.memset(x1[:], 0.0)
        nc.vector.memset(x2[:], 0.0)
        nc.vector.memset(pf[:].bitcast(F32), 0.0)
        nc.vector.memset(qflat[12][:].bitcast(F32), 0.0)

        onesh = rowsh[:, H_ONES:H_ONES + 512]

        # ================= phase A: conv1d stack + heads =================
        with tc.tile_pool(name="convw", bufs=1) as pw, \
             tc.tile_pool(name="convp", bufs=4, space="PSUM") as pp:

            def wtile(tag, shape):
                return pw.tile(shape, F16, tag=tag, name=tag)

            xp = [wtile(f"xp{k}", [128, W]) for k in range(4)]
            for kt in range(4):
                nc.gpsimd.dma_start(xp[kt][:], io["xp"][kt * 128:(kt + 1) * 128, :])
            wb1 = wtile("wb1", [128, 12 * 256])
            nc.sync.dma_start(wb1[:], io["wb1T"])
            w2 = {}
            rings = [nc.scalar, nc.sync, nc.scalar, nc.sync]
            for ri, nm in enumerate(("wb2T", "ws1T", "we1T", "wpT")):
                w2[nm] = wtile(nm, [128, 6 * 256])
                rings[ri].dma_start(w2[nm][:], io[nm])
            wse = wtile("wse", [128, 4])
            nc.gpsimd.dma_start(wse[:], io["wseT"])

            h1 = [wtile(f"h1{i}", [128, W]) for i in range(2)]
            h = [wtile(f"h{i}", [128, W]) for i in range(2)]
            s_sb = [wtile(f"s{i}", [128, T]) for i in range(2)]
            e_sb = [wtile(f"e{i}", [128, T]) for i in range(2)]
            for i in range(2):
                nc.vector.memset(h1[i][:].bitcast(F32), 0.0)
                nc.vector.memset(h[i][:].bitcast(F32), 0.0)

            def conv1d(src_tiles, w_sb, nkt, bias_off, dst_tiles, dst_pad):
                for ot in range(2):
                    ps = pp.tile([128, T], F32, tag="cvp", name="cvp")
                    first = True
                    for k in range(3):
                        for kt in range(nkt):
                            lhsT = w_sb[:, (k * nkt + kt) * 256 + ot * 128:
                                        (k * nkt + kt) * 256 + ot * 128 + 128]
                            _mm(nc, ps[:], lhsT, src_tiles[kt][:, k:k + T],
                                first, False)
                            first = False
                    _mm(nc, ps[:], rowsh[:, bias_off + ot * 128:bias_off + ot * 128 + 128],
                        onesh[:, 0:T], False, True)
                    if dst_pad:
                        relu(dst_tiles[ot][:, 1:T + 1], ps[:])
                    else:
                        relu(dst_tiles[ot][:, 0:T], ps[:])

            conv1d(xp, wb1, 4, H_BB1, h1, True)
            conv1d(h1, w2["wb2T"], 2, H_BB2, h, True)
            conv1d(h, w2["ws1T"], 2, H_BS1, s_sb, False)
            conv1d(h, w2["we1T"], 2, H_BE1, e_sb, False)
            conv1d(h, w2["wpT"], 2, H_BP,
                   [pf[:].rearrange("p (two m) -> p two m", two=2)[:, 0, :],
                    pf[:].rearrange("p (two m) -> p two m", two=2)[:, 1, :]],
                   False)

            # heads: sigmoid(w . s + b)
            for col, src, dst in ((0, s_sb, ost), (1, e_sb, oen)):
                ph = pp.tile([1, T], F32, tag="cvh", name="cvh")
                for kt in range(2):
                    _mm(nc, ph[:], wse[:, kt * 2 + col:kt * 2 + col + 1],
                        src[kt][:, 0:T], kt == 0, False)
                _mm(nc, ph[:], rowsh[:, H_BSE + col:H_BSE + col + 1],
                    onesh[:, 0:T], False, True)
                nc.scalar.activation(dst[:], ph[:], AF.Sigmoid)
            nc.sync.dma_start(io["out_start"], ost[:])
            nc.sync.dma_start(io["out_end"], oen[:])

        # ================= phase B: qT + flat repack =================
        p_mk = tc.alloc_tile_pool(name="mpool", bufs=7)
        p_ps = tc.alloc_tile_pool(name="bcpsum", bufs=1, space="PSUM")
        mk_cc0 = []

        for (a, b) in ((0, 5), (5, 9), (9, 13)):
            mk = p_mk.tile([128, 5 * 1024], F8, tag="mk", name="mk")
            nc.gpsimd.dma_start(mk[:, 0:(b - a) * 1024],
                                io["maskf"][0, :, a * 1024:b * 1024])
            mk_cc0.append(mk)
        pfv = pf[:].rearrange("p (two m) -> p two m", two=2)
        with tc.tile_pool(name="w3pool", bufs=3) as p_w3, \
             tc.tile_pool(name="qtmp", bufs=8) as p_qt:
            for g in range(4):
                w3a = p_w3.tile([128, 16 * H3], F8, tag="w3", name="w3a")
                nc.sync.dma_start(w3a[:], io["w3dT"][g])
                w3v = w3a[:].rearrange("p (ni two o) -> p ni two o",
                                       ni=8, two=2)
                for ni in range(8):
                    n = 8 * g + ni
                    qp = p_ps.tile([128, H3], F32, tag="qp", name="qp", bufs=2)
                    nc.tensor.matmul(qp[:], pfv, w3v[:, ni, :, :],
                                     start=True, stop=True, perf_mode=DR)
                    qt = p_qt.tile([T, H3], F8, tag="qt", name="qt")
                    if n % 2 == 0:
                        nc.vector.tensor_copy(qt[:], qp[0:T, :])
                    else:
                        nc.scalar.copy(qt[:], qp[0:T, :])
                    # scatter rows [100n, 100n+100) into (chain, plane) tiles
                    g0 = n * T
                    r = 0
                    while r < T:
                        kt, off = divmod(g0 + r, 128)
                        kt2, plane = divmod(kt, 2)
                        cnt = min(T - r, 128 - off)
                        nc.gpsimd.dma_start(
                            qflat[kt2][off:off + cnt,
                                       plane * H3:plane * H3 + H3],
                            qt[r:r + cnt, :])
                        r += cnt

        nc.sync.dma_start(valid[:], io["valid"])
        nc.sync.dma_start(vmask2[:], io["vmask2"])
        nc.sync.dma_start(wq1s[:], io["wq1T"])
        nc.sync.dma_start(wq2s[:], io["wq2T"])
        nc.sync.dma_start(wq3s[:], io["wq3T"])
        nc.sync.dma_start(wq4s[:], io["wq4T"])

        # ================= phase C: M2' + wq1, per column block =================
        with tc.tile_pool(name="ypool", bufs=2) as p_y:
            for cc, (c0, csz) in enumerate(CC):
                yt = []
                mps = [p_ps.tile([128, csz], F32, tag=f"mp{ot}", name=f"mp{ot}",
                                 bufs=1) for ot in range(4)]
                for gi, (a, b) in enumerate(((0, 5), (5, 9), (9, 13))):
                    if cc == 0:
                        mk = mk_cc0[gi]
                    else:
                        mk = p_mk.tile([128, 5 * 1024], F8, tag="mk", name="mk")
                        nc.sync.dma_start(
                            mk[:, 0:(b - a) * 1024],
                            io["maskf"][cc, :, a * 1024:b * 1024])
                    mkv = mk[:].rearrange("p (l two c) -> p l two c",
                                          two=2, c=512)
                    for sub in range(b - a):
                        kt2 = a + sub
                        rhs = mkv[:, sub, :, 0:csz]
                        for ot in range(4):
                            qv = qflat[kt2][:].rearrange(
                                "p (two o) -> p two o", two=2)
                            nc.tensor.matmul(
                                mps[ot][:], qv[:, :, ot * 128:(ot + 1) * 128],
                                rhs, start=(kt2 == 0), stop=False,
                                perf_mode=DR)
                for ot in range(4):
                    _mm(nc, mps[ot][:], rowsh[:, H_B3D + ot * 128:H_B3D + ot * 128 + 128],
                        valid[:, c0:c0 + csz], False, True)
                    y = p_y.tile([128, 500], F16, tag=f"y{ot}", name=f"y{ot}")
                    relu(y[:, 0:csz], mps[ot][:])
                    yt.append(y)
                # wq1 1x1 -> x1 (padded strided dest)
                q1 = p_ps.tile([128, csz], F32, tag="q1", name="q1", bufs=1)
                for kt in range(4):
                    _mm(nc, q1[:], wq1s[:, kt * 128:(kt + 1) * 128],
                        yt[kt][:, 0:csz], kt == 0, False)
                _mm(nc, q1[:], rowsh[:, H_BQ1:H_BQ1 + 128],
                    valid[:, c0:c0 + csz], False, True)
                r0, nr = c0 // T, csz // T
                relu(x1[:, _Fi(r0, 1):_Fi(r0, 1) + nr * W]
                     .rearrange("p (r w) -> p r w", w=W)[:, :, 0:T],
                     q1[:].rearrange("p (r w) -> p r w", w=T))

        p_ps.release()
        p_mk.release()

        # ================= phase D: wq2 -> x2 (masked) =================
        WIN2 = ((1, 5), (6, 5), (11, 5), (16, 5), (21, 5), (26, 2))
        with tc.tile_pool(name="c2psum", bufs=1, space="PSUM") as pc2, \
             tc.tile_pool(name="x3pool", bufs=2) as p_x3, \
             tc.tile_pool(name="c4psum", bufs=2, space="PSUM") as pc4:
            ps2 = [pc2.tile([128, 510], F32, tag=f"c2w{wi}", name=f"c2w{wi}")
                   for wi in range(6)]
            for d in range(9):
                di, dj = d // 3 - 1, d % 3 - 1
                for wi, (r0, nr) in enumerate(WIN2):
                    wsz = nr * W
                    off = _Fi(r0, 0) + di * W + dj
                    _mm(nc, ps2[wi][:, 0:wsz], wq2s[:, d * 128:(d + 1) * 128],
                        x1[:, off:off + wsz], d == 0, False)
            for wi, (r0, nr) in enumerate(WIN2):
                wsz = nr * W
                _mm(nc, ps2[wi][:, 0:wsz], rowsh[:, H_BQ2:H_BQ2 + 128],
                    onesh[:, 0:wsz], False, True)
                # fused relu * validity-mask, strided into x2
                nc.vector.scalar_tensor_tensor(
                    x2[:, _Fi(r0, 1):_Fi(r0, 1) + wsz]
                    .rearrange("p (r w) -> p r w", w=W)[:, :, 0:T],
                    ps2[wi][:, 0:wsz].rearrange("p (r w) -> p r w", w=W)[:, :, 1:T + 1],
                    0.0,
                    vmask2[:, (r0 - 1) * T:(r0 - 1 + nr) * T]
                    .rearrange("p (r w) -> p r w", w=T),
                    ALU.max, ALU.mult)

            # ============= phase E: wq3 + wq4 + sigmoid out =============
            ps3 = [pc2.tile([128, 510], F32, tag=f"c2w{wi}", name=f"c3w{wi}")
                   for wi in range(5)]
            for d in range(9):
                di, dj = d // 3 - 1, d % 3 - 1
                for bi, r0 in enumerate((2, 7, 12, 17, 22)):
                    off = _Fi(r0, 0) + di * W + dj
                    _mm(nc, ps3[bi][:, 0:510], wq3s[:, d * 128:(d + 1) * 128],
                        x2[:, off:off + 510], d == 0, False)
            for bi, r0 in enumerate((2, 7, 12, 17, 22)):
                wsz = 5 * W
                ps = ps3[bi]
                _mm(nc, ps[:, 0:wsz], rowsh[:, H_BQ3:H_BQ3 + 128],
                    onesh[:, 0:wsz], False, True)
                x3 = p_x3.tile([128, 5 * T], F16, tag="x3", name="x3")
                relu(x3[:].rearrange("p (r w) -> p r w", w=T),
                     ps[:, 0:wsz].rearrange("p (r w) -> p r w", w=W)[:, :, 1:T + 1])
                p4 = pc4.tile([2, 5 * T], F32, tag="c4", name="c4")
                _mm(nc, p4[:], wq4s[:, 0:2], x3[:], True, False)
                _mm(nc, p4[:], rowsh[:, H_BQ4:H_BQ4 + 2], onesh[:, 0:5 * T],
                    False, True)
                nc.scalar.activation(ocm[:, bi * 5 * T:(bi + 1) * 5 * T], p4[:],
                                     AF.Sigmoid)
            nc.scalar.dma_start(io["out_cm"], ocm[:])


_CACHE = {}


def _build():
    if "nc" not in _CACHE:
        nc = bacc.Bacc("TRN2", target_bir_lowering=False, debug=False)
        io = {}
        for name, shape in DRAM_IN:
            dt = F32 if name == "vmask2" else (F8 if name in ("maskf", "w3dT") else F16)
            io[name] = nc.dram_tensor(name, list(shape), dt,
                                      kind="ExternalInput").ap()
        for name, shape in DRAM_OUT:
            io[name] = nc.dram_tensor(name, list(shape), F32,
                                      kind="ExternalOutput").ap()
        with tile.TileContext(nc) as tc:
            _emit(tc, io)
        nc.compile()
        _CACHE["nc"] = nc
    return _CACHE["nc"]


# ---------------- host-side prep ----------------

def _dense_grouped(w, pad_to):
    # returns [128, (k kt) * 256] sbuf-image: lhsT slices at (k*nkt+kt)*256+o
    out_c, cin_g, K = w.shape
    og = out_c // G
    dense = np.zeros((K, pad_to, out_c), np.float16)
    for o in range(out_c):
        g = o // og
        dense[:, g * cin_g:(g + 1) * cin_g, o] = w[o].T
    nkt = pad_to // 128
    # [k, kt*128+p, o] -> [p, (k, kt), o]
    return np.ascontiguousarray(
        dense.reshape(K, nkt, 128, out_c).transpose(2, 0, 1, 3)
        .reshape(128, K * nkt * out_c))


def _host_prep(inputs):
    pr = {}
    pr["sm"] = np.ascontiguousarray(
        np.asarray(inputs["sample_mask"], np.float32).reshape(T, N, T, T))
    pr["wb1T"] = _dense_grouped(np.asarray(inputs["wb1"], np.float16), 512)
    pr["wb2T"] = _dense_grouped(np.asarray(inputs["wb2"], np.float16), 256)
    pr["ws1T"] = _dense_grouped(np.asarray(inputs["ws1"], np.float16), 256)
    pr["we1T"] = _dense_grouped(np.asarray(inputs["we1"], np.float16), 256)
    wpT = np.asarray(inputs["wp"], np.float16).transpose(2, 1, 0)  # [3,256,256]
    pr["wpT"] = np.ascontiguousarray(
        wpT.reshape(3, 2, 128, 256).transpose(2, 0, 1, 3).reshape(128, 6 * 256))
    wseT = np.stack([np.asarray(inputs["ws2"], np.float16)[0, :, 0],
                     np.asarray(inputs["we2"], np.float16)[0, :, 0]], axis=1)
    pr["wseT"] = np.ascontiguousarray(
        wseT.reshape(2, 128, 2).transpose(1, 0, 2).reshape(128, 4))
    rowsh = np.zeros((1, ROWSH_LEN), np.float16)
    rowsh[0, H_B3D:H_B3D + 512] = np.asarray(inputs["b3d"], np.float16)
    rowsh[0, H_BQ1:H_BQ1 + 128] = np.asarray(inputs["bq1"], np.float16)
    rowsh[0, H_BQ2:H_BQ2 + 128] = np.asarray(inputs["bq2"], np.float16)
    rowsh[0, H_BQ3:H_BQ3 + 128] = np.asarray(inputs["bq3"], np.float16)
    rowsh[0, H_BQ4:H_BQ4 + 2] = np.asarray(inputs["bq4"], np.float16)
    rowsh[0, H_ONES:H_ONES + 512] = 1.0
    for off, key in ((H_BB1, "bb1"), (H_BB2, "bb2"), (H_BS1, "bs1"),
                     (H_BE1, "be1"), (H_BP, "bp")):
        rowsh[0, off:off + 256] = np.asarray(inputs[key], np.float16)
    rowsh[0, H_BSE] = np.float16(np.asarray(inputs["bs2"])[0])
    rowsh[0, H_BSE + 1] = np.float16(np.asarray(inputs["be2"])[0])
    pr["rowsh"] = rowsh
    w3d = np.asarray(inputs["w3d"], np.float32)
    np8 = mybir.dt.np(F8)
    w3t = w3d.transpose(1, 2, 0).reshape(2, 128, 4, 8, H3)  # [plane,p,g,ni,o]
    pr["w3dT"] = np.ascontiguousarray(
        w3t.transpose(2, 1, 3, 0, 4).reshape(4, 128, 16 * H3).astype(np8))
    wq1T = np.asarray(inputs["wq1"], np.float16)[:, :, 0, 0].T  # [512,128]
    pr["wq1T"] = np.ascontiguousarray(
        wq1T.reshape(4, 128, 128).transpose(1, 0, 2).reshape(128, 512))
    wq2T = np.asarray(inputs["wq2"], np.float16).transpose(2, 3, 1, 0).reshape(9, H2, H2)
    pr["wq2T"] = np.ascontiguousarray(
        wq2T.transpose(1, 0, 2).reshape(128, 9 * 128))
    wq3T = np.asarray(inputs["wq3"], np.float16).transpose(2, 3, 1, 0).reshape(9, H2, H2)
    pr["wq3T"] = np.ascontiguousarray(
        wq3T.transpose(1, 0, 2).reshape(128, 9 * 128))
    pr["wq4T"] = np.ascontiguousarray(
        np.asarray(inputs["wq4"], np.float16)[:, :, 0, 0].T)
    return pr


def _core_inputs(inputs, pr, core):
    b, iq = divmod(core, 4)
    lo = RPC * iq
    ci = {}
    xp = np.zeros((512, W), np.float16)
    xp[:C_IN, 1:T + 1] = np.asarray(inputs["x"], np.float16)[b].T
    ci["xp"] = xp
    msk = np.zeros((N, T, REXT, T), np.float32)
    rlo, rhi = max(0, 2 - lo), min(REXT, T + 2 - lo)
    msk[:, :, rlo:rhi, :] = pr["sm"][:, :, lo - 2 + rlo:lo - 2 + rhi, :] \
        .transpose(1, 0, 2, 3)
    np8 = mybir.dt.np(F8)
    mf = np.zeros((13, 2, 128, REXT * T), np.float32)
    mf.reshape(13 * 2 * 128, REXT * T)[:N * T] = msk.reshape(N * T, REXT * T)
    mg = np.zeros((6, 128, 13, 2, 512), np8)
    for cc, (c0, csz) in enumerate(CC):
        mg[cc, :, :, :, :csz] = \
            mf[:, :, :, c0:c0 + csz].transpose(2, 0, 1, 3).astype(np8)
    ci["maskf"] = np.ascontiguousarray(mg.reshape(6, 128, 13 * 2 * 512))
    vr = np.zeros((REXT,), np.float32)
    vr[rlo:rhi] = 1.0
    ci["valid"] = np.repeat(vr, T)[None, :].astype(np.float16)
    ci["vmask2"] = np.broadcast_to(
        np.repeat(vr[1:28], T)[None, :], (128, 27 * T)).copy()
    for k in ("wb1T", "wb2T", "ws1T", "we1T", "wpT", "wseT", "rowsh",
              "w3dT", "wq1T", "wq2T", "wq3T", "wq4T"):
        ci[k] = pr[k]
    return ci


def _run(inputs, **spmd_kwargs):
    nc = _build()
    pr = _host_prep(inputs)
    in_maps = [_core_inputs(inputs, pr, c) for c in range(NCORES)]
    res = run_bass_kernel_spmd(nc, in_maps, core_ids=list(range(NCORES)),
                               **spmd_kwargs)
    cm = np.zeros((B, 2, T, T), np.float32)
    start = np.zeros((B, T), np.float32)
    end = np.zeros((B, T), np.float32)
    for core in range(NCORES):
        b, iq = divmod(core, 4)
        r = res.results[core]
        cm[b, :, RPC * iq:RPC * (iq + 1), :] = r["out_cm"].reshape(2, RPC, T)
        if iq == 0:
            start[b] = r["out_start"][0]
            end[b] = r["out_end"][0]
    return (cm, start, end), res


def kernel(**inputs):
    out, _ = _run(inputs)
    return out
